# revision 17
# baseline (speedup 1.0000x reference)
"""RNN-T Joiner kernel for 8x TRN2 NeuronCores (Bass/Tile).

out[b,t,u,v] = (enc[b,t]@W_enc.T + b_enc) @ W1.T
            + (pred[b,u]@W_pred.T + b_pred) @ W2.T + b_out
with W1 = W_out[:, :J], W2 = W_out[:, J:].

Strategy: data-parallel over batch (B=8 == n_cores). All biases fold into a
single vector c[v] = W1@b_enc + W2@b_pred + b_out (host-side). Inputs are
host-cast to fp16 and pre-tiled to [128, k*C] so each is one contiguous DMA.
Per core:
  S1: E^T[j,t], P^T[j,u] via fp16 PE matmuls (1 cyc/row).
  S2: PbC[u,v] = P@W2.T + c (K=1 ones-matmul folds c into the PSUM group);
      dPbC[u] = PbC[u] - PbC[u-1] (DVE, fp16 deltas; dPbC[0] = PbC[0]).
  S3 delta-accumulation: one persistent PSUM tile per t-block. Chain start
      computes Ev = E@W1.T directly into PSUM; then per u a single K=1
      ones-matmul accumulates dPbC[u], so PSUM always holds Ev + PbC[u] --
      each output column is written exactly once per u by the PE. DVE (tb0)
      and ACT (tb1) copy PSUM -> fp16 out tiles; 2MB DMAs stream to HBM.
      Output is fp16 on the wire (rel err ~1e-3 vs the 2e-2 gate); host
      upcasts to f32.
"""

import numpy as np

ENC_DIM, DEC_DIM, J, V = 512, 640, 512, 1024
B, T, U = 8, 256, 64
N_CORES = 8
UBLK = 8  # u-values per output tile / DMA ([128, UBLK*1024] fp16 = 2MB DMA)
BURST = 16  # dep-free PE warm-up matmuls per u-group (HAM re-warm burst)
GROUPS = [2, 2, 4, 8, 8, 8, 8, 8, 8, 4, 2, 2]  # u's per output tile/DMA
assert sum(GROUPS) == U

NE = ENC_DIM // 128  # 4
ND = DEC_DIM // 128  # 5
NJ = J // 128        # 4
NT = T // 128        # 2
NV = V // 512        # 2

_CACHE: dict = {}


def _ensure_path():
    try:
        import concourse.bass  # noqa: F401
    except ImportError:
        import sys

        for p in ("/opt/trn_rl_repo", "/root/.axon_site/_ro/trn_rl_repo"):
            if p not in sys.path:
                sys.path.insert(0, p)


def _build_nc():
    import concourse.mybir as mybir
    from concourse import bacc
    from concourse.tile import TileContext

    f16 = mybir.dt.float16
    f32 = mybir.dt.float32
    nc = bacc.Bacc("TRN2", target_bir_lowering=False, debug=False,
                   num_devices=N_CORES)

    enc_d = nc.dram_tensor("enc_s", [128, NE * T], f16, kind="ExternalInput")
    pred_d = nc.dram_tensor("pred_s", [128, ND * U], f16, kind="ExternalInput")
    wenc_d = nc.dram_tensor("wenc_s", [128, NE * J], f16, kind="ExternalInput")
    wpred_d = nc.dram_tensor("wpred_s", [128, ND * J], f16, kind="ExternalInput")
    w1_d = nc.dram_tensor("w1_s", [128, NJ * V], f16, kind="ExternalInput")
    w2_d = nc.dram_tensor("w2_s", [128, NJ * V], f16, kind="ExternalInput")
    cvec_d = nc.dram_tensor("cvec", [1, V], f16, kind="ExternalInput")
    out_d = nc.dram_tensor("out", [T, U * V], f16, kind="ExternalOutput")

    with TileContext(nc) as tc:
        with (
            tc.tile_pool(name="const", bufs=1) as const,
            tc.tile_pool(name="ot0", bufs=3) as opool0,
            tc.tile_pool(name="ot1", bufs=3) as opool1,
            tc.tile_pool(name="ps_small", bufs=2, space="PSUM") as ps_s,
            tc.tile_pool(name="ps_pers", bufs=1, space="PSUM") as ps_p,
        ):
            # e0[0, m] = 1 iff m == 0: lhsT that adds cvec only to row u=0.
            e0 = const.tile([1, U], f16, tag="e0", name="e0")
            nc.gpsimd.memset(e0[:, :], 0.0)
            nc.gpsimd.memset(e0[0:1, 0:1], 1.0)
            # sel[k, u*128+m] = 1 if k == u else 0: sel[:, u*128:(u+1)*128] is
            # the lhsT that broadcasts dPbC row u across all 128 partitions.
            sel = const.tile([U, U * 128], f16, tag="sel", name="sel")
            nc.gpsimd.memset(sel[:, :], 0.0)
            nc.gpsimd.affine_select(
                out=sel[:, :].rearrange("p (u m) -> p u m", m=128),
                in_=sel[:, :].rearrange("p (u m) -> p u m", m=128),
                compare_op=mybir.AluOpType.not_equal,
                fill=1.0,
                base=0,
                pattern=[[-1, U], [0, 128]],
                channel_multiplier=1,
            )
            cvec = const.tile([1, V], f16, tag="cvec", name="cvec")

            def load(tag, dram, cols):
                t = const.tile([128, cols], f16, tag=tag, name=tag)
                nc.sync.dma_start(t[:, :], dram.ap()[:, :])
                return t

            # Load order follows the critical path to the first output store:
            # S2b/dPbC need pred+wpred+w2+cvec; chain starts need enc+wenc+w1
            # (w1 split in halves so chain matmuls m=0,1 overlap the m=2,3
            # arrival).
            nc.sync.dma_start(cvec[:, :], cvec_d.ap()[:, :])
            preds = load("pred", pred_d, ND * U)
            wpred = load("wpred", wpred_d, ND * J)
            w2 = load("w2_", w2_d, NJ * V)
            encs = load("enc", enc_d, NE * T)
            wenc = load("wenc", wenc_d, NE * J)
            w1 = const.tile([128, NJ * V], f16, tag="w1_", name="w1_")
            nc.sync.dma_start(w1[:, :2 * V], w1_d.ap()[:, :2 * V])
            nc.sync.dma_start(w1[:, 2 * V:], w1_d.ap()[:, 2 * V:])

            # S1b: P^T[j,u] in 4 chunks of [128, 64]
            PT = []
            for m in range(NJ):
                ps = ps_s.tile([128, U], f32, tag="s1", name="ps1")
                for c in range(ND):
                    nc.tensor.matmul(
                        ps[:, :],
                        lhsT=wpred[:, c * J + m * 128:c * J + (m + 1) * 128],
                        rhs=preds[:, c * U:(c + 1) * U],
                        start=(c == 0), stop=(c == ND - 1))
                t = const.tile([128, U], f16, tag=f"PT{m}", name=f"PT{m}")
                nc.vector.tensor_copy(t[:, :], ps[:, :])
                PT.append(t)

            # S1a: E^T[j,t] in 4 chunks of [128, 256]
            ET = []
            for m in range(NJ):
                ps = ps_s.tile([128, T], f32, tag="s1", name="ps1")
                for c in range(NE):
                    nc.tensor.matmul(
                        ps[:, :],
                        lhsT=wenc[:, c * J + m * 128:c * J + (m + 1) * 128],
                        rhs=encs[:, c * T:(c + 1) * T],
                        start=(c == 0), stop=(c == NE - 1))
                t = const.tile([128, T], f16, tag=f"ET{m}", name=f"ET{m}")
                nc.vector.tensor_copy(t[:, :], ps[:, :])
                ET.append(t)

            # delta over u of P^T (free-dim shift, so base partition stays 0):
            # PTd[:, 0] = PT[:, 0]; PTd[:, u] = PT[:, u] - PT[:, u-1]
            PTd = []
            for m in range(NJ):
                t = const.tile([128, U], f16, tag=f"PTd{m}", name=f"PTd{m}")
                nc.vector.tensor_copy(t[:, 0:1], PT[m][:, 0:1])
                nc.vector.tensor_sub(t[:, 1:U], PT[m][:, 1:U], PT[m][:, 0:U - 1])
                PTd.append(t)

            # S2b on deltas: dPbC[u] = (P^T delta_u) @ W2.T (+ c only at u=0),
            # i.e. dPbC[0] = Pb[0]+c and dPbC[u] = Pb[u] - Pb[u-1].
            dPbC = const.tile([U, V], f16, tag="dPbC", name="dPbC")
            for vb in range(NV):
                ps = ps_s.tile([U, 512], f32, tag="s1", name="ps1")
                for m in range(NJ):
                    nc.tensor.matmul(ps[:, :], lhsT=PTd[m][:, :],
                                     rhs=w2[:, m * V + vb * 512:m * V + (vb + 1) * 512],
                                     start=(m == 0), stop=False)
                nc.tensor.matmul(ps[:, :], lhsT=e0[:, :],
                                 rhs=cvec[:, vb * 512:(vb + 1) * 512],
                                 start=False, stop=True)
                nc.vector.tensor_copy(dPbC[:, vb * 512:(vb + 1) * 512], ps[:, :])

            # S3: four independent persistent PSUM chains, one per (tb, vb).
            # Each starts as Ev = E@W1.T (fused S2a), then accumulates
            # dPbC[u] every step so it always holds Ev + PbC[u]. Chains are
            # separate [128,512] banks so the copy of one chain overlaps the
            # accumulates of the others.
            CH = [(tb, vb) for tb in range(NT) for vb in range(NV)]
            pst = {c: ps_p.tile([128, 512], f32, tag=f"pst{c[0]}{c[1]}",
                                name=f"pst{c[0]}{c[1]}") for c in CH}
            scr = ps_p.tile([128, 512], f32, tag="scr", name="scr")
            for tb, vb in CH:
                for m in range(NJ):
                    nc.tensor.matmul(
                        pst[(tb, vb)][:, :],
                        lhsT=ET[m][:, tb * 128:(tb + 1) * 128],
                        rhs=w1[:, m * V + vb * 512:m * V + (vb + 1) * 512],
                        start=(m == 0), stop=False)

            # Graduated group sizes: small groups at the start stream the
            # first stores ~10us earlier (no 8-u pipeline-fill stall) and
            # small groups at the end shrink the drain tail to one 0.5MB DMA.
            u0 = 0
            for gsz in GROUPS:
                ot = [opool0.tile([128, gsz * V], f16, tag="ot0", name="ot0"),
                      opool1.tile([128, gsz * V], f16, tag="ot1", name="ot1")]
                for uu in range(gsz):
                    u = u0 + uu
                    last = (u == U - 1)
                    selu = sel[:, u * 128:(u + 1) * 128]
                    for tb, vb in CH:
                        sl = slice(vb * 512, (vb + 1) * 512)
                        nc.tensor.matmul(pst[(tb, vb)][:, :],
                                         lhsT=selu, rhs=dPbC[:, sl],
                                         start=False, stop=last)
                    for tb, vb in CH:
                        dst = ot[tb][:, uu * V + vb * 512:uu * V + (vb + 1) * 512]
                        if vb == 0:
                            nc.vector.tensor_copy(dst, pst[(tb, vb)][:, :])
                        else:
                            nc.scalar.copy(dst, pst[(tb, vb)][:, :])
                for tb in range(NT):
                    nc.sync.dma_start(
                        out_d.ap()[tb * 128:(tb + 1) * 128,
                                   u0 * V:(u0 + gsz) * V],
                        ot[tb][:, :])
                u0 += gsz
                if gsz == 8 and u0 < U:
                    # Dep-free back-to-back PE burst (~3.5us warm): forces a
                    # fully-busy HAM activity window so the PE clock re-arms
                    # to 2.4 GHz. The burst hides under this group's ~11us of
                    # copy/DMA work (reads consts, writes a scratch bank).
                    for _ in range(BURST):
                        nc.tensor.matmul(scr[:, :], lhsT=sel[:, 0:128],
                                         rhs=dPbC[:, 0:512],
                                         start=True, stop=True)
    nc.compile()
    return nc


def _get_nc():
    if "nc" not in _CACHE:
        _ensure_path()
        _CACHE["nc"] = _build_nc()
    return _CACHE["nc"]


def _tile128(a, nchunks):
    """[nchunks*128, C] -> [128, nchunks*C] fp16 (SBUF 128-partition layout)."""
    r, c = a.shape
    assert r == nchunks * 128
    return np.ascontiguousarray(
        a.reshape(nchunks, 128, c).transpose(1, 0, 2).reshape(128, nchunks * c)
    ).astype(np.float16)


def _prep_in_maps(enc_out, pred_out, W_enc, b_enc, W_pred, b_pred, W_out, b_out):
    f = np.float32
    enc_out = np.asarray(enc_out, f)
    pred_out = np.asarray(pred_out, f)
    W_enc = np.asarray(W_enc, f)
    W_pred = np.asarray(W_pred, f)
    W_out = np.asarray(W_out, f)
    W1, W2 = W_out[:, :J], W_out[:, J:]
    cvec = (W1 @ np.asarray(b_enc, f) + W2 @ np.asarray(b_pred, f)
            + np.asarray(b_out, f)).astype(np.float16)[None, :]
    shared = {
        "wenc_s": _tile128(W_enc.T, NE),
        "wpred_s": _tile128(W_pred.T, ND),
        "w1_s": _tile128(np.ascontiguousarray(W1.T), NJ),
        "w2_s": _tile128(np.ascontiguousarray(W2.T), NJ),
        "cvec": cvec,
    }
    return [
        {"enc_s": _tile128(enc_out[b].T, NE),
         "pred_s": _tile128(pred_out[b].T, ND), **shared}
        for b in range(B)
    ]


def run(in_maps, trace=False, **kw):
    _ensure_path()
    from concourse.bass_utils import run_bass_kernel_spmd

    return run_bass_kernel_spmd(_get_nc(), in_maps, list(range(N_CORES)),
                                trace=trace, **kw)


def kernel(enc_out, pred_out, W_enc, b_enc, W_pred, b_pred, W_out, b_out):
    in_maps = _prep_in_maps(enc_out, pred_out, W_enc, b_enc, W_pred, b_pred,
                            W_out, b_out)
    res = run(in_maps, trace=False)
    return np.stack([r["out"].astype(np.float32).reshape(T, U, V)
                     for r in res.results], axis=0)


# revision 24
# speedup vs baseline: 1.0294x; 1.0294x over previous
"""RNN-T Joiner kernel for 8x TRN2 NeuronCores (Bass/Tile).

out[b,t,u,v] = (enc[b,t]@W_enc.T + b_enc) @ W1.T
            + (pred[b,u]@W_pred.T + b_pred) @ W2.T + b_out
with W1 = W_out[:, :J], W2 = W_out[:, J:].

Strategy: data-parallel over batch (B=8 == n_cores). All biases fold into a
single vector c[v] = W1@b_enc + W2@b_pred + b_out (host-side). Inputs are
host-cast to fp16 and pre-tiled to [128, k*C] so each is one contiguous DMA.
Per core:
  S1: E^T[j,t], P^T[j,u] via fp16 PE matmuls (1 cyc/row).
  S2: PbC[u,v] = P@W2.T + c (K=1 ones-matmul folds c into the PSUM group);
      dPbC[u] = PbC[u] - PbC[u-1] (DVE, fp16 deltas; dPbC[0] = PbC[0]).
  S3 delta-accumulation: one persistent PSUM tile per t-block. Chain start
      computes Ev = E@W1.T directly into PSUM; then per u a single K=1
      ones-matmul accumulates dPbC[u], so PSUM always holds Ev + PbC[u] --
      each output column is written exactly once per u by the PE. DVE (tb0)
      and ACT (tb1) copy PSUM -> fp16 out tiles; 2MB DMAs stream to HBM.
      Output is fp16 on the wire (rel err ~1e-3 vs the 2e-2 gate); host
      upcasts to f32.
"""

import numpy as np

ENC_DIM, DEC_DIM, J, V = 512, 640, 512, 1024
B, T, U = 8, 256, 64
N_CORES = 8
UBLK = 8  # u-values per output tile / DMA ([128, UBLK*1024] fp16 = 2MB DMA)
BURST = 16  # dep-free PE warm-up matmuls per u-group (HAM re-warm burst)
GROUPS = [2, 2, 4, 8, 8, 8, 8, 8, 8, 4, 2, 2]  # u's per output tile/DMA
assert sum(GROUPS) == U

NE = ENC_DIM // 128  # 4
ND = DEC_DIM // 128  # 5
NJ = J // 128        # 4
NT = T // 128        # 2
NV = V // 512        # 2

_CACHE: dict = {}


def _ensure_path():
    try:
        import concourse.bass  # noqa: F401
    except ImportError:
        import sys

        for p in ("/opt/trn_rl_repo", "/root/.axon_site/_ro/trn_rl_repo"):
            if p not in sys.path:
                sys.path.insert(0, p)


def _build_nc():
    import concourse.mybir as mybir
    from concourse import bacc
    from concourse.tile import TileContext

    f16 = mybir.dt.float16
    f32 = mybir.dt.float32
    nc = bacc.Bacc("TRN2", target_bir_lowering=False, debug=False,
                   num_devices=N_CORES)

    enc_d = nc.dram_tensor("enc_s", [128, NE * T], f16, kind="ExternalInput")
    pred_d = nc.dram_tensor("pred_s", [128, ND * U], f16, kind="ExternalInput")
    wenc_d = nc.dram_tensor("wenc_s", [128, NE * J], f16, kind="ExternalInput")
    wpred_d = nc.dram_tensor("wpred_s", [128, ND * J], f16, kind="ExternalInput")
    w1_d = nc.dram_tensor("w1_s", [128, NJ * V], f16, kind="ExternalInput")
    w2_d = nc.dram_tensor("w2_s", [128, NJ * V], f16, kind="ExternalInput")
    cvec_d = nc.dram_tensor("cvec", [1, V], f16, kind="ExternalInput")
    out_d = nc.dram_tensor("out", [T, U * V], f16, kind="ExternalOutput")

    with TileContext(nc) as tc:
        with (
            tc.tile_pool(name="const", bufs=1) as const,
            tc.tile_pool(name="ot0", bufs=3) as opool0,
            tc.tile_pool(name="ot1", bufs=3) as opool1,
            tc.tile_pool(name="ps_small", bufs=2, space="PSUM") as ps_s,
            tc.tile_pool(name="ps_pers", bufs=1, space="PSUM") as ps_p,
        ):
            # e0[0, m] = 1 iff m == 0: lhsT that adds cvec only to row u=0.
            e0 = const.tile([1, U], f16, tag="e0", name="e0")
            nc.gpsimd.memset(e0[:, :], 0.0)
            nc.gpsimd.memset(e0[0:1, 0:1], 1.0)
            # sel[k, u*128+m] = 1 if k == u else 0: sel[:, u*128:(u+1)*128] is
            # the lhsT that broadcasts dPbC row u across all 128 partitions.
            # Built in two parts so block u=0 (needed by the first S3 step)
            # is ready ~10us before the 8us affine_select for the rest lands.
            sel = const.tile([U, U * 128], f16, tag="sel", name="sel")
            U0 = 8

            def build_sel(lo, hi):
                part = sel[:, lo * 128:hi * 128]
                nc.gpsimd.memset(part, 0.0)
                nc.gpsimd.affine_select(
                    out=part.rearrange("p (u m) -> p u m", m=128),
                    in_=part.rearrange("p (u m) -> p u m", m=128),
                    compare_op=mybir.AluOpType.not_equal,
                    fill=1.0,
                    base=-lo,
                    pattern=[[-1, hi - lo], [0, 128]],
                    channel_multiplier=1,
                )

            build_sel(0, U0)
            cvec = const.tile([1, V], f16, tag="cvec", name="cvec")

            def load(tag, dram, cols):
                t = const.tile([128, cols], f16, tag=tag, name=tag)
                nc.sync.dma_start(t[:, :], dram.ap()[:, :])
                return t

            # Loads split across three DMA paths so nothing waits ~14us on a
            # single FIFO: SP ring (sync) takes the S1 weights, ACT ring
            # (scalar) takes cvec/enc/w2, SWDGE (gpsimd) takes w1. gpsimd
            # issues w1 between the two sel parts (before the 8us part-2).
            def loadq(q, tag, dram, cols):
                t = const.tile([128, cols], f16, tag=tag, name=tag)
                q.dma_start(t[:, :], dram.ap()[:, :])
                return t

            preds = loadq(nc.sync, "pred", pred_d, ND * U)
            wpred = loadq(nc.sync, "wpred", wpred_d, ND * J)
            wenc = loadq(nc.sync, "wenc", wenc_d, NE * J)
            nc.scalar.dma_start(cvec[:, :], cvec_d.ap()[:, :])
            encs = loadq(nc.scalar, "enc", enc_d, NE * T)
            w2 = loadq(nc.scalar, "w2_", w2_d, NJ * V)
            w1 = loadq(nc.sync, "w1_", w1_d, NJ * V)
            build_sel(U0, U)

            # S1b: P^T[j,u] in 4 chunks of [128, 64]
            PT = []
            for m in range(NJ):
                ps = ps_s.tile([128, U], f32, tag="s1", name="ps1")
                for c in range(ND):
                    nc.tensor.matmul(
                        ps[:, :],
                        lhsT=wpred[:, c * J + m * 128:c * J + (m + 1) * 128],
                        rhs=preds[:, c * U:(c + 1) * U],
                        start=(c == 0), stop=(c == ND - 1))
                t = const.tile([128, U], f16, tag=f"PT{m}", name=f"PT{m}")
                nc.vector.tensor_copy(t[:, :], ps[:, :])
                PT.append(t)

            # S1a: E^T[j,t] in 4 chunks of [128, 256]
            ET = []
            for m in range(NJ):
                ps = ps_s.tile([128, T], f32, tag="s1", name="ps1")
                for c in range(NE):
                    nc.tensor.matmul(
                        ps[:, :],
                        lhsT=wenc[:, c * J + m * 128:c * J + (m + 1) * 128],
                        rhs=encs[:, c * T:(c + 1) * T],
                        start=(c == 0), stop=(c == NE - 1))
                t = const.tile([128, T], f16, tag=f"ET{m}", name=f"ET{m}")
                if m % 2 == 0:
                    nc.vector.tensor_copy(t[:, :], ps[:, :])
                else:
                    nc.scalar.copy(t[:, :], ps[:, :])
                ET.append(t)

            # delta over u of P^T (free-dim shift, so base partition stays 0):
            # PTd[:, 0] = PT[:, 0]; PTd[:, u] = PT[:, u] - PT[:, u-1]
            PTd = []
            for m in range(NJ):
                t = const.tile([128, U], f16, tag=f"PTd{m}", name=f"PTd{m}")
                nc.vector.tensor_copy(t[:, 0:1], PT[m][:, 0:1])
                nc.vector.tensor_sub(t[:, 1:U], PT[m][:, 1:U], PT[m][:, 0:U - 1])
                PTd.append(t)

            # S2b on deltas: dPbC[u] = (P^T delta_u) @ W2.T (+ c only at u=0),
            # i.e. dPbC[0] = Pb[0]+c and dPbC[u] = Pb[u] - Pb[u-1].
            dPbC = const.tile([U, V], f16, tag="dPbC", name="dPbC")
            for vb in range(NV):
                ps = ps_s.tile([U, 512], f32, tag="s1", name="ps1")
                for m in range(NJ):
                    nc.tensor.matmul(ps[:, :], lhsT=PTd[m][:, :],
                                     rhs=w2[:, m * V + vb * 512:m * V + (vb + 1) * 512],
                                     start=(m == 0), stop=False)
                nc.tensor.matmul(ps[:, :], lhsT=e0[:, :],
                                 rhs=cvec[:, vb * 512:(vb + 1) * 512],
                                 start=False, stop=True)
                nc.scalar.copy(dPbC[:, vb * 512:(vb + 1) * 512], ps[:, :])

            # S3: four independent persistent PSUM chains, one per (tb, vb).
            # Each starts as Ev = E@W1.T (fused S2a), then accumulates
            # dPbC[u] every step so it always holds Ev + PbC[u]. Chains are
            # separate [128,512] banks so the copy of one chain overlaps the
            # accumulates of the others.
            CH = [(tb, vb) for tb in range(NT) for vb in range(NV)]
            pst = {c: ps_p.tile([128, 512], f32, tag=f"pst{c[0]}{c[1]}",
                                name=f"pst{c[0]}{c[1]}") for c in CH}
            scr = ps_p.tile([128, 512], f32, tag="scr", name="scr")
            for tb, vb in CH:
                for m in range(NJ):
                    nc.tensor.matmul(
                        pst[(tb, vb)][:, :],
                        lhsT=ET[m][:, tb * 128:(tb + 1) * 128],
                        rhs=w1[:, m * V + vb * 512:m * V + (vb + 1) * 512],
                        start=(m == 0), stop=False)

            # Graduated group sizes: small groups at the start stream the
            # first stores ~10us earlier (no 8-u pipeline-fill stall) and
            # small groups at the end shrink the drain tail to one 0.5MB DMA.
            u0 = 0
            for gsz in GROUPS:
                ot = [opool0.tile([128, gsz * V], f16, tag="ot0", name="ot0"),
                      opool1.tile([128, gsz * V], f16, tag="ot1", name="ot1")]
                for uu in range(gsz):
                    u = u0 + uu
                    last = (u == U - 1)
                    selu = sel[:, u * 128:(u + 1) * 128]
                    for tb, vb in CH:
                        sl = slice(vb * 512, (vb + 1) * 512)
                        nc.tensor.matmul(pst[(tb, vb)][:, :],
                                         lhsT=selu, rhs=dPbC[:, sl],
                                         start=False, stop=last)
                    for tb, vb in CH:
                        dst = ot[tb][:, uu * V + vb * 512:uu * V + (vb + 1) * 512]
                        if vb == 0:
                            nc.vector.tensor_copy(dst, pst[(tb, vb)][:, :])
                        else:
                            nc.scalar.copy(dst, pst[(tb, vb)][:, :])
                for tb in range(NT):
                    nc.sync.dma_start(
                        out_d.ap()[tb * 128:(tb + 1) * 128,
                                   u0 * V:(u0 + gsz) * V],
                        ot[tb][:, :])
                u0 += gsz
                if gsz == 8 and u0 < U:
                    # Dep-free back-to-back PE burst (~3.5us warm): forces a
                    # fully-busy HAM activity window so the PE clock re-arms
                    # to 2.4 GHz. The burst hides under this group's ~11us of
                    # copy/DMA work (reads consts, writes a scratch bank).
                    for _ in range(BURST):
                        nc.tensor.matmul(scr[:, :], lhsT=sel[:, 0:128],
                                         rhs=dPbC[:, 0:512],
                                         start=True, stop=True)
    nc.compile()
    return nc


def _get_nc():
    if "nc" not in _CACHE:
        _ensure_path()
        _CACHE["nc"] = _build_nc()
    return _CACHE["nc"]


def _tile128(a, nchunks):
    """[nchunks*128, C] -> [128, nchunks*C] fp16 (SBUF 128-partition layout)."""
    r, c = a.shape
    assert r == nchunks * 128
    return np.ascontiguousarray(
        a.reshape(nchunks, 128, c).transpose(1, 0, 2).reshape(128, nchunks * c)
    ).astype(np.float16)


def _prep_in_maps(enc_out, pred_out, W_enc, b_enc, W_pred, b_pred, W_out, b_out):
    f = np.float32
    enc_out = np.asarray(enc_out, f)
    pred_out = np.asarray(pred_out, f)
    W_enc = np.asarray(W_enc, f)
    W_pred = np.asarray(W_pred, f)
    W_out = np.asarray(W_out, f)
    W1, W2 = W_out[:, :J], W_out[:, J:]
    cvec = (W1 @ np.asarray(b_enc, f) + W2 @ np.asarray(b_pred, f)
            + np.asarray(b_out, f)).astype(np.float16)[None, :]
    shared = {
        "wenc_s": _tile128(W_enc.T, NE),
        "wpred_s": _tile128(W_pred.T, ND),
        "w1_s": _tile128(np.ascontiguousarray(W1.T), NJ),
        "w2_s": _tile128(np.ascontiguousarray(W2.T), NJ),
        "cvec": cvec,
    }
    return [
        {"enc_s": _tile128(enc_out[b].T, NE),
         "pred_s": _tile128(pred_out[b].T, ND), **shared}
        for b in range(B)
    ]


def run(in_maps, trace=False, **kw):
    _ensure_path()
    from concourse.bass_utils import run_bass_kernel_spmd

    return run_bass_kernel_spmd(_get_nc(), in_maps, list(range(N_CORES)),
                                trace=trace, **kw)


def kernel(enc_out, pred_out, W_enc, b_enc, W_pred, b_pred, W_out, b_out):
    in_maps = _prep_in_maps(enc_out, pred_out, W_enc, b_enc, W_pred, b_pred,
                            W_out, b_out)
    res = run(in_maps, trace=False)
    return np.stack([r["out"].astype(np.float32).reshape(T, U, V)
                     for r in res.results], axis=0)


# revision 25
# speedup vs baseline: 1.0561x; 1.0259x over previous
"""RNN-T Joiner kernel for 8x TRN2 NeuronCores (Bass/Tile).

out[b,t,u,v] = (enc[b,t]@W_enc.T + b_enc) @ W1.T
            + (pred[b,u]@W_pred.T + b_pred) @ W2.T + b_out
with W1 = W_out[:, :J], W2 = W_out[:, J:].

Strategy: data-parallel over batch (B=8 == n_cores). All biases fold into a
single vector c[v] = W1@b_enc + W2@b_pred + b_out (host-side). Inputs are
host-cast to fp16 and pre-tiled to [128, k*C] so each is one contiguous DMA.
Per core:
  S1: E^T[j,t], P^T[j,u] via fp16 PE matmuls (1 cyc/row).
  S2: PbC[u,v] = P@W2.T + c (K=1 ones-matmul folds c into the PSUM group);
      dPbC[u] = PbC[u] - PbC[u-1] (DVE, fp16 deltas; dPbC[0] = PbC[0]).
  S3 delta-accumulation: one persistent PSUM tile per t-block. Chain start
      computes Ev = E@W1.T directly into PSUM; then per u a single K=1
      ones-matmul accumulates dPbC[u], so PSUM always holds Ev + PbC[u] --
      each output column is written exactly once per u by the PE. DVE (tb0)
      and ACT (tb1) copy PSUM -> fp16 out tiles; 2MB DMAs stream to HBM.
      Output is fp16 on the wire (rel err ~1e-3 vs the 2e-2 gate); host
      upcasts to f32.
"""

import numpy as np

ENC_DIM, DEC_DIM, J, V = 512, 640, 512, 1024
B, T, U = 8, 256, 64
N_CORES = 8
UBLK = 8  # u-values per output tile / DMA ([128, UBLK*1024] fp16 = 2MB DMA)
BURST = 16  # dep-free PE warm-up matmuls per u-group (HAM re-warm burst)
GROUPS = [2, 2, 4, 8, 8, 8, 8, 8, 8, 4, 2, 2]  # u's per output tile/DMA
assert sum(GROUPS) == U

NE = ENC_DIM // 128  # 4
ND = DEC_DIM // 128  # 5
NJ = J // 128        # 4
NT = T // 128        # 2
NV = V // 512        # 2

_CACHE: dict = {}


def _ensure_path():
    try:
        import concourse.bass  # noqa: F401
    except ImportError:
        import sys

        for p in ("/opt/trn_rl_repo", "/root/.axon_site/_ro/trn_rl_repo"):
            if p not in sys.path:
                sys.path.insert(0, p)


def _build_nc():
    import concourse.mybir as mybir
    from concourse import bacc
    from concourse.tile import TileContext

    f16 = mybir.dt.float16
    f32 = mybir.dt.float32
    nc = bacc.Bacc("TRN2", target_bir_lowering=False, debug=False,
                   num_devices=N_CORES)

    enc_d = nc.dram_tensor("enc_s", [128, NE * T], f16, kind="ExternalInput")
    pred_d = nc.dram_tensor("pred_s", [128, ND * U], f16, kind="ExternalInput")
    wenc_d = nc.dram_tensor("wenc_s", [128, NE * J], f16, kind="ExternalInput")
    wpred_d = nc.dram_tensor("wpred_s", [128, ND * J], f16, kind="ExternalInput")
    w1_d = nc.dram_tensor("w1_s", [128, NJ * V], f16, kind="ExternalInput")
    w2_d = nc.dram_tensor("w2_s", [128, NJ * V], f16, kind="ExternalInput")
    cvec_d = nc.dram_tensor("cvec", [1, V], f16, kind="ExternalInput")
    out_d = nc.dram_tensor("out", [T, U * V], f16, kind="ExternalOutput")

    with TileContext(nc) as tc:
        with (
            tc.tile_pool(name="const", bufs=1) as const,
            tc.tile_pool(name="ot0", bufs=3) as opool0,
            tc.tile_pool(name="ot1", bufs=3) as opool1,
            tc.tile_pool(name="ps_small", bufs=2, space="PSUM") as ps_s,
            tc.tile_pool(name="ps_pers", bufs=1, space="PSUM") as ps_p,
        ):
            # e0[0, m] = 1 iff m == 0: lhsT that adds cvec only to row u=0.
            e0 = const.tile([1, U], f16, tag="e0", name="e0")
            nc.gpsimd.memset(e0[:, :], 0.0)
            nc.gpsimd.memset(e0[0:1, 0:1], 1.0)
            # sel[k, u*128+m] = 1 if k == u else 0: sel[:, u*128:(u+1)*128] is
            # the lhsT that broadcasts dPbC row u across all 128 partitions.
            # Built in two parts so block u=0 (needed by the first S3 step)
            # is ready ~10us before the 8us affine_select for the rest lands.
            sel = const.tile([U, U * 128], f16, tag="sel", name="sel")
            U0 = 8

            def build_sel(lo, hi):
                part = sel[:, lo * 128:hi * 128]
                nc.gpsimd.memset(part, 0.0)
                nc.gpsimd.affine_select(
                    out=part.rearrange("p (u m) -> p u m", m=128),
                    in_=part.rearrange("p (u m) -> p u m", m=128),
                    compare_op=mybir.AluOpType.not_equal,
                    fill=1.0,
                    base=-lo,
                    pattern=[[-1, hi - lo], [0, 128]],
                    channel_multiplier=1,
                )

            build_sel(0, U0)
            cvec = const.tile([1, V], f16, tag="cvec", name="cvec")

            def load(tag, dram, cols):
                t = const.tile([128, cols], f16, tag=tag, name=tag)
                nc.sync.dma_start(t[:, :], dram.ap()[:, :])
                return t

            # Loads split across three DMA paths so nothing waits ~14us on a
            # single FIFO: SP ring (sync) takes the S1 weights, ACT ring
            # (scalar) takes cvec/enc/w2, SWDGE (gpsimd) takes w1. gpsimd
            # issues w1 between the two sel parts (before the 8us part-2).
            def loadq(q, tag, dram, cols):
                t = const.tile([128, cols], f16, tag=tag, name=tag)
                q.dma_start(t[:, :], dram.ap()[:, :])
                return t

            preds = loadq(nc.sync, "pred", pred_d, ND * U)
            wpred = loadq(nc.sync, "wpred", wpred_d, ND * J)
            wenc = loadq(nc.sync, "wenc", wenc_d, NE * J)
            nc.scalar.dma_start(cvec[:, :], cvec_d.ap()[:, :])
            encs = loadq(nc.scalar, "enc", enc_d, NE * T)
            w2 = loadq(nc.scalar, "w2_", w2_d, NJ * V)
            w1 = loadq(nc.sync, "w1_", w1_d, NJ * V)
            build_sel(U0, U)

            # Early PE warm-up: dep-free matmuls on data that lands first
            # (e0 memset ~7us, cvec is the first scalar-ring load ~7.5us).
            # PE is otherwise idle until ~12.5us; ~3.4us of contiguous busy
            # flips the HAM clock gate to 2.4 GHz, so all prologue matmuls
            # (S1/S2/chain starts, ~16us cold) run warm instead.
            scr_w = ps_p.tile([U, 512], f32, tag="scrw", name="scrw")
            for _ in range(12):
                nc.tensor.matmul(scr_w[:, :], lhsT=e0[:, :],
                                 rhs=cvec[:, 0:512], start=True, stop=True)

            # S1b: P^T[j,u] in 4 chunks of [128, 64]
            PT = []
            for m in range(NJ):
                ps = ps_s.tile([128, U], f32, tag="s1", name="ps1")
                for c in range(ND):
                    nc.tensor.matmul(
                        ps[:, :],
                        lhsT=wpred[:, c * J + m * 128:c * J + (m + 1) * 128],
                        rhs=preds[:, c * U:(c + 1) * U],
                        start=(c == 0), stop=(c == ND - 1))
                t = const.tile([128, U], f16, tag=f"PT{m}", name=f"PT{m}")
                nc.vector.tensor_copy(t[:, :], ps[:, :])
                PT.append(t)

            # S1a: E^T[j,t] in 4 chunks of [128, 256]
            ET = []
            for m in range(NJ):
                ps = ps_s.tile([128, T], f32, tag="s1", name="ps1")
                for c in range(NE):
                    nc.tensor.matmul(
                        ps[:, :],
                        lhsT=wenc[:, c * J + m * 128:c * J + (m + 1) * 128],
                        rhs=encs[:, c * T:(c + 1) * T],
                        start=(c == 0), stop=(c == NE - 1))
                t = const.tile([128, T], f16, tag=f"ET{m}", name=f"ET{m}")
                if m % 2 == 0:
                    nc.vector.tensor_copy(t[:, :], ps[:, :])
                else:
                    nc.scalar.copy(t[:, :], ps[:, :])
                ET.append(t)

            # delta over u of P^T (free-dim shift, so base partition stays 0):
            # PTd[:, 0] = PT[:, 0]; PTd[:, u] = PT[:, u] - PT[:, u-1]
            PTd = []
            for m in range(NJ):
                t = const.tile([128, U], f16, tag=f"PTd{m}", name=f"PTd{m}")
                nc.vector.tensor_copy(t[:, 0:1], PT[m][:, 0:1])
                nc.vector.tensor_sub(t[:, 1:U], PT[m][:, 1:U], PT[m][:, 0:U - 1])
                PTd.append(t)

            # S2b on deltas: dPbC[u] = (P^T delta_u) @ W2.T (+ c only at u=0),
            # i.e. dPbC[0] = Pb[0]+c and dPbC[u] = Pb[u] - Pb[u-1].
            dPbC = const.tile([U, V], f16, tag="dPbC", name="dPbC")
            for vb in range(NV):
                ps = ps_s.tile([U, 512], f32, tag="s1", name="ps1")
                for m in range(NJ):
                    nc.tensor.matmul(ps[:, :], lhsT=PTd[m][:, :],
                                     rhs=w2[:, m * V + vb * 512:m * V + (vb + 1) * 512],
                                     start=(m == 0), stop=False)
                nc.tensor.matmul(ps[:, :], lhsT=e0[:, :],
                                 rhs=cvec[:, vb * 512:(vb + 1) * 512],
                                 start=False, stop=True)
                nc.scalar.copy(dPbC[:, vb * 512:(vb + 1) * 512], ps[:, :])

            # S3: four independent persistent PSUM chains, one per (tb, vb).
            # Each starts as Ev = E@W1.T (fused S2a), then accumulates
            # dPbC[u] every step so it always holds Ev + PbC[u]. Chains are
            # separate [128,512] banks so the copy of one chain overlaps the
            # accumulates of the others.
            CH = [(tb, vb) for tb in range(NT) for vb in range(NV)]
            pst = {c: ps_p.tile([128, 512], f32, tag=f"pst{c[0]}{c[1]}",
                                name=f"pst{c[0]}{c[1]}") for c in CH}
            scr = ps_p.tile([128, 512], f32, tag="scr", name="scr")
            for tb, vb in CH:
                for m in range(NJ):
                    nc.tensor.matmul(
                        pst[(tb, vb)][:, :],
                        lhsT=ET[m][:, tb * 128:(tb + 1) * 128],
                        rhs=w1[:, m * V + vb * 512:m * V + (vb + 1) * 512],
                        start=(m == 0), stop=False)

            # Graduated group sizes: small groups at the start stream the
            # first stores ~10us earlier (no 8-u pipeline-fill stall) and
            # small groups at the end shrink the drain tail to one 0.5MB DMA.
            u0 = 0
            for gsz in GROUPS:
                ot = [opool0.tile([128, gsz * V], f16, tag="ot0", name="ot0"),
                      opool1.tile([128, gsz * V], f16, tag="ot1", name="ot1")]
                for uu in range(gsz):
                    u = u0 + uu
                    last = (u == U - 1)
                    selu = sel[:, u * 128:(u + 1) * 128]
                    for tb, vb in CH:
                        sl = slice(vb * 512, (vb + 1) * 512)
                        nc.tensor.matmul(pst[(tb, vb)][:, :],
                                         lhsT=selu, rhs=dPbC[:, sl],
                                         start=False, stop=last)
                    for tb, vb in CH:
                        dst = ot[tb][:, uu * V + vb * 512:uu * V + (vb + 1) * 512]
                        if vb == 0:
                            nc.vector.tensor_copy(dst, pst[(tb, vb)][:, :])
                        else:
                            nc.scalar.copy(dst, pst[(tb, vb)][:, :])
                for tb in range(NT):
                    nc.sync.dma_start(
                        out_d.ap()[tb * 128:(tb + 1) * 128,
                                   u0 * V:(u0 + gsz) * V],
                        ot[tb][:, :])
                u0 += gsz
                if gsz == 8 and u0 < U:
                    # Dep-free back-to-back PE burst (~3.5us warm): forces a
                    # fully-busy HAM activity window so the PE clock re-arms
                    # to 2.4 GHz. The burst hides under this group's ~11us of
                    # copy/DMA work (reads consts, writes a scratch bank).
                    for _ in range(BURST):
                        nc.tensor.matmul(scr[:, :], lhsT=sel[:, 0:128],
                                         rhs=dPbC[:, 0:512],
                                         start=True, stop=True)
    nc.compile()
    return nc


def _get_nc():
    if "nc" not in _CACHE:
        _ensure_path()
        _CACHE["nc"] = _build_nc()
    return _CACHE["nc"]


def _tile128(a, nchunks):
    """[nchunks*128, C] -> [128, nchunks*C] fp16 (SBUF 128-partition layout)."""
    r, c = a.shape
    assert r == nchunks * 128
    return np.ascontiguousarray(
        a.reshape(nchunks, 128, c).transpose(1, 0, 2).reshape(128, nchunks * c)
    ).astype(np.float16)


def _prep_in_maps(enc_out, pred_out, W_enc, b_enc, W_pred, b_pred, W_out, b_out):
    f = np.float32
    enc_out = np.asarray(enc_out, f)
    pred_out = np.asarray(pred_out, f)
    W_enc = np.asarray(W_enc, f)
    W_pred = np.asarray(W_pred, f)
    W_out = np.asarray(W_out, f)
    W1, W2 = W_out[:, :J], W_out[:, J:]
    cvec = (W1 @ np.asarray(b_enc, f) + W2 @ np.asarray(b_pred, f)
            + np.asarray(b_out, f)).astype(np.float16)[None, :]
    shared = {
        "wenc_s": _tile128(W_enc.T, NE),
        "wpred_s": _tile128(W_pred.T, ND),
        "w1_s": _tile128(np.ascontiguousarray(W1.T), NJ),
        "w2_s": _tile128(np.ascontiguousarray(W2.T), NJ),
        "cvec": cvec,
    }
    return [
        {"enc_s": _tile128(enc_out[b].T, NE),
         "pred_s": _tile128(pred_out[b].T, ND), **shared}
        for b in range(B)
    ]


def run(in_maps, trace=False, **kw):
    _ensure_path()
    from concourse.bass_utils import run_bass_kernel_spmd

    return run_bass_kernel_spmd(_get_nc(), in_maps, list(range(N_CORES)),
                                trace=trace, **kw)


def kernel(enc_out, pred_out, W_enc, b_enc, W_pred, b_pred, W_out, b_out):
    in_maps = _prep_in_maps(enc_out, pred_out, W_enc, b_enc, W_pred, b_pred,
                            W_out, b_out)
    res = run(in_maps, trace=False)
    return np.stack([r["out"].astype(np.float32).reshape(T, U, V)
                     for r in res.results], axis=0)
